# revision 3
# baseline (speedup 1.0000x reference)
"""GRU layer (flax GRUCell math) on 8 Trainium2 NeuronCores — v2.

Data-parallel: batch 64 sharded 8-way (8 rows/core); weights replicated;
the T=4096 recurrence runs locally per core.

v2 changes vs baseline:
- sigmoid via tanh identity with host-side 0.5-scaled weights: PSUM holds
  s/2, gates come from tanh only; r = (tanh+1)/2 folded into downstream ops.
- n-gate: v = (tr+1)*pnh_half via one DVE scalar_tensor_tensor.
- h' = n*u1 + q via plain DVE ops with contiguous APs; u1 and q computed
  in the shadow of other chain links.
- the 0.5*h' staging runs on Pool (GpSimd), off critical path.
- Output staging holds 0.5*h'; epilogue copy scales by 2.
- PE per-step order r,n,z so ACT(tr) and DVE(v) start as early as possible.
- chunk prep (DMA/transpose/GEMM pieces) interleaved as thunks between scan
  steps to bound PE/ACT/DVE blocking.
"""

import sys

sys.path.insert(0, "/opt/trn_rl_repo")

import numpy as np

import concourse.bacc as bacc
import concourse.tile as tile
from concourse import mybir
from concourse.masks import make_identity
from concourse.bass_utils import run_bass_kernel_spmd

F32 = mybir.dt.float32
BF16 = mybir.dt.bfloat16
AF = mybir.ActivationFunctionType
ALU = mybir.AluOpType

B, T, D, H = 64, 4096, 128, 128
NCORES = 8
BL = B // NCORES  # 8 batch rows per core


def tri(ap, j):
    """Stride-3 view of a [P, 3*BL] tile: column j of each per-b triple."""
    return ap.rearrange("p (b j) -> p b j", j=3)[:, :, j]


def build_gru_nc(BL=BL, T=T, C=64, R=1):
    assert T % C == 0
    BT = C * BL
    assert (BT * 4) % 2048 == 0, "gate region must be a whole PSUM bank"
    NCH = T // C
    NBLK = BT // 128
    TBLK = 128 // BL

    nc = bacc.Bacc("TRN2", target_bir_lowering=False, debug=False)

    x_d = nc.dram_tensor("x", [BL, T, D], F32, kind="ExternalInput").ap()
    # packed [z | r | n]: wi = [Wiz/2 | Wir/2 | Win], wh = [Whz/2 | Whr/2 | Whn/2]
    wi_d = nc.dram_tensor("wi", [D, 3 * H], BF16, kind="ExternalInput").ap()
    wh_d = nc.dram_tensor("wh", [H, 3 * H], BF16, kind="ExternalInput").ap()
    # b_row = [b_iz/2 | b_ir/2 | b_hn/2 | b_in]
    brow_d = nc.dram_tensor("b_row", [1, 4 * H], F32, kind="ExternalInput").ap()
    bin_d = nc.dram_tensor("b_in", [H, 1], F32, kind="ExternalInput").ap()
    y_d = nc.dram_tensor("y", [BL, T, H], F32, kind="ExternalOutput").ap()

    x_tbd = x_d.rearrange("b t d -> t b d")
    y_tbh = y_d.rearrange("b t h -> t b h")

    with tile.TileContext(nc) as tc:
        with (
            tc.tile_pool(name="const", bufs=1) as const_p,
            tc.tile_pool(name="xraw", bufs=2 * NBLK) as xraw_p,
            tc.tile_pool(name="xt", bufs=2) as xt_p,
            tc.tile_pool(name="gn", bufs=2) as gn_p,
            tc.tile_pool(name="hs", bufs=2) as hs_p,
            tc.tile_pool(name="outt", bufs=2 * NBLK) as outt_p,
            tc.tile_pool(name="small", bufs=8) as small_p,
            tc.tile_pool(name="prz", bufs=2, space="PSUM") as prz_p,
            tc.tile_pool(name="pb", bufs=2, space="PSUM") as pb_p,
            tc.tile_pool(name="pscr", bufs=2, space="PSUM") as pscr_p,
        ):
            wi = const_p.tile([D, 3 * H], BF16)
            nc.sync.dma_start(wi[:], wi_d)
            wh = const_p.tile([H, 3 * H], BF16)
            nc.sync.dma_start(wh[:], wh_d)
            brow = const_p.tile([1, 4 * H], F32)
            nc.sync.dma_start(brow[:], brow_d)
            bin_ = const_p.tile([H, 1], F32)
            nc.sync.dma_start(bin_[:], bin_d)
            ones = const_p.tile([1, BT], F32)
            nc.vector.memset(ones[:], 1.0)
            ident = const_p.tile([128, 128], F32)
            make_identity(nc, ident[:])
            hinit = const_p.tile([H, BL], BF16)
            nc.vector.memset(hinit[:], 0.0)
            halfs = const_p.tile([H, BL], F32)
            nc.vector.memset(halfs[:], 0.5)

            NROT = 4
            et = [const_p.tile([H, BL], BF16, tag=f"et{k}", name=f"et{k}") for k in range(NROT)]
            qq = [const_p.tile([H, BL], BF16, tag=f"qq{k}", name=f"qq{k}") for k in range(NROT)]
            # step 0 reads q=0 from qq[0] before any DVE write touches it
            nc.vector.memset(qq[0][:], 0.0)

            def make_prep(c):
                """Thunk list preparing chunk c's x projections."""
                t0 = c * C
                xt = xt_p.tile([D, BT], BF16, tag="xt")
                prz = prz_p.tile([128, 2 * BT], F32, tag="prz")
                pb = pb_p.tile([128, BT], F32, tag="pb")
                gn = gn_p.tile([128, BT], F32, tag="gn")
                thunks = []
                xrs = []

                def dma_piece(k):
                    xr = xraw_p.tile([128, 128], F32, tag="xraw")
                    xrs.append(xr)
                    nc.sync.dma_start(
                        xr[:], x_tbd[t0 + TBLK * k : t0 + TBLK * (k + 1)]
                    )

                def tp_piece(k):
                    ps = pscr_p.tile([128, BT], F32, tag="scr")
                    nc.tensor.transpose(ps[:, 0:128], xrs[k][:], ident[:])
                    nc.scalar.activation(
                        xt[:, 128 * k : 128 * (k + 1)], ps[:, 0:128], AF.Copy
                    )

                def bias_piece():
                    nc.tensor.matmul(prz[:, 0:BT], brow[:, 0:H], ones[:], start=True, stop=False, skip_group_check=True)
                    nc.tensor.matmul(prz[:, BT : 2 * BT], brow[:, H : 2 * H], ones[:], start=True, stop=False, skip_group_check=True)
                    nc.tensor.matmul(pb[:], brow[:, 2 * H : 3 * H], ones[:], start=True, stop=False, skip_group_check=True)
                    gn_ps.append(pscr_p.tile([128, BT], F32, tag="scr", name="gnps"))
                    nc.tensor.matmul(gn_ps[0][:], brow[:, 3 * H : 4 * H], ones[:], start=True, stop=False, skip_group_check=True)

                def gemm_piece(g, j):
                    # g: 0=z (cols 0:BT), 1=r (cols BT:2BT)
                    sl = slice(g * BT + 128 * j, g * BT + 128 * (j + 1))
                    nc.tensor.matmul(
                        prz[:, sl], wi[:, g * H : (g + 1) * H],
                        xt[:, 128 * j : 128 * (j + 1)], start=False, stop=False,
                        skip_group_check=True,
                    )

                gn_ps = []

                def gn_gemm_piece(j):
                    nc.tensor.matmul(
                        gn_ps[0][:, 128 * j : 128 * (j + 1)], wi[:, 2 * H : 3 * H],
                        xt[:, 128 * j : 128 * (j + 1)],
                        start=False, stop=True, skip_group_check=True,
                    )

                def gn_act_piece(j):
                    nc.scalar.activation(
                        gn[:, 128 * j : 128 * (j + 1)],
                        gn_ps[0][:, 128 * j : 128 * (j + 1)],
                        AF.Identity,
                    )

                for k in range(NBLK):
                    thunks.append(lambda k=k: dma_piece(k))
                for k in range(NBLK):
                    thunks.append(lambda k=k: tp_piece(k))
                thunks.append(bias_piece)
                for j in range(NBLK):
                    thunks.append(lambda j=j: gemm_piece(0, j))
                for j in range(NBLK):
                    thunks.append(lambda j=j: gemm_piece(1, j))
                for j in range(NBLK):
                    thunks.append(lambda j=j: gn_gemm_piece(j))
                for j in range(NBLK):
                    thunks.append(lambda j=j: gn_act_piece(j))
                return thunks, (xt, prz, pb, gn)

            def make_epilogue(stage, t0):
                thunks = []

                def ep_piece(k):
                    ps = pscr_p.tile([128, BT], F32, tag="scr")
                    nc.tensor.transpose(ps[:, 0:128], stage[:, 128 * k : 128 * (k + 1)], ident[:])
                    ot = outt_p.tile([128, 128], F32, tag="outt")
                    nc.vector.tensor_scalar_mul(ot[:], ps[:, 0:128], 2.0)
                    nc.sync.dma_start(y_tbh[t0 + TBLK * k : t0 + TBLK * (k + 1)], ot[:])

                for k in range(NBLK):
                    thunks.append(lambda k=k: ep_piece(k))
                return thunks

            # prologue: fully emit chunk 0 prep, then start chunk 1 prep queue
            pr0, cur_bufs = make_prep(0)
            for th in pr0:
                th()

            pending: list = []
            prev_stage = None
            chunk_seq = [(rep, c) for rep in range(R) for c in range(NCH)]
            for ci, (rep, c) in enumerate(chunk_seq):
                t0 = c * C
                xt, prz, pb, gn = cur_bufs
                if ci + 1 < len(chunk_seq):
                    nxt_thunks, nxt_bufs = make_prep(chunk_seq[ci + 1][1])
                else:
                    nxt_thunks, nxt_bufs = [], None
                pending = pending + nxt_thunks

                stage = hs_p.tile([H, BT], F32, tag="hs")

                for tl in range(C):
                    t = rep * T + t0 + tl
                    k = t % NROT
                    ET, QT = et[k], qq[k]
                    cs = slice(tl * BL, (tl + 1) * BL)
                    rcs = slice(BT + tl * BL, BT + (tl + 1) * BL)

                    if t == 0:
                        qrhs, erhs = qq[0][:], hinit[:]
                    else:
                        qrhs = qq[(t - 1) % NROT][:]
                        erhs = et[(t - 1) % NROT][:]

                    # PE: q-side matmuls first (q ready early; no stop), then
                    # e-side (r, n, z order; carries the stop)
                    nc.tensor.matmul(prz[:, rcs], wh[:, H : 2 * H], qrhs, start=False, stop=False, skip_group_check=True)
                    nc.tensor.matmul(pb[:, cs], wh[:, 2 * H : 3 * H], qrhs, start=False, stop=False, skip_group_check=True)
                    nc.tensor.matmul(prz[:, cs], wh[:, 0:H], qrhs, start=False, stop=False, skip_group_check=True)
                    nc.tensor.matmul(prz[:, rcs], wh[:, H : 2 * H], erhs, start=False, stop=True, skip_group_check=True)
                    nc.tensor.matmul(pb[:, cs], wh[:, 2 * H : 3 * H], erhs, start=False, stop=True, skip_group_check=True)
                    nc.tensor.matmul(prz[:, cs], wh[:, 0:H], erhs, start=False, stop=True, skip_group_check=True)

                    trt = small_p.tile([H, BL], F32, tag="tr")
                    nc.scalar.activation(trt[:], prz[:, rcs], AF.Tanh)
                    tzt = small_p.tile([H, BL], F32, tag="tz")
                    nc.scalar.activation(tzt[:], prz[:, cs], AF.Tanh)

                    v = small_p.tile([H, BL], F32, tag="v")
                    nc.vector.scalar_tensor_tensor(
                        v[:], trt[:], 1.0, pb[:, cs], op0=ALU.add, op1=ALU.mult
                    )
                    w = small_p.tile([H, BL], F32, tag="w")
                    nc.vector.tensor_add(w[:], v[:], gn[:, cs])
                    # shadow of ACT(n): u1 = 0.5 - 0.5*tz ; q = (tz+1)*(0.5*h_prev)
                    u1 = small_p.tile([H, BL], F32, tag="u1")
                    nc.vector.scalar_tensor_tensor(
                        u1[:], tzt[:], -0.5, halfs[:], op0=ALU.mult, op1=ALU.add
                    )
                    if t > 0:
                        ptl = tl - 1
                        if ptl >= 0:
                            pcol = stage[:, ptl * BL : (ptl + 1) * BL]
                        else:
                            pcol = prev_stage[:, (C - 1) * BL : C * BL]
                        nc.vector.scalar_tensor_tensor(
                            QT[:], tzt[:], 1.0, pcol, op0=ALU.add, op1=ALU.mult
                        )
                    nt = small_p.tile([H, BL], F32, tag="nt")
                    nc.scalar.activation(nt[:], w[:], AF.Tanh)
                    nc.vector.tensor_mul(ET[:], nt[:], u1[:])
                    # Pool (off-path): h' = e + q, stage <- 0.5 * h'
                    s1 = small_p.tile([H, BL], F32, tag="s1")
                    nc.gpsimd.tensor_add(s1[:], ET[:], QT[:])
                    nc.gpsimd.tensor_mul(stage[:, cs], s1[:], halfs[:])

                    # interleave one pending prep/epilogue thunk every 2 steps
                    if tl % 2 == 1 and pending:
                        pending.pop(0)()

                # any prep not yet emitted for next chunk: flush now
                while pending:
                    pending.pop(0)()
                pending = make_epilogue(stage, t0)
                prev_stage = stage
                cur_bufs = nxt_bufs

            while pending:
                pending.pop(0)()

    nc.compile()
    return nc


_NC_CACHE = {}


def _get_nc(BL_, T_, C_):
    key = (BL_, T_, C_)
    if key not in _NC_CACHE:
        _NC_CACHE[key] = build_gru_nc(BL_, T_, C_)
    return _NC_CACHE[key]


def pack_weights(Wir, Wiz, Win, Whr, Whz, Whn, b_ir, b_iz, b_in, b_hn):
    """Pack + pre-scale weights for the kernel's [z | r | n] layout."""
    import jax.numpy as _jnp
    wi = np.ascontiguousarray(
        np.asarray(
            _jnp.asarray(
                np.concatenate([0.5 * Wiz, 0.5 * Wir, Win], axis=1),
                dtype=_jnp.bfloat16,
            )
        )
    )
    import jax.numpy as jnp
    wh = np.ascontiguousarray(
        np.asarray(
            jnp.asarray(
                np.concatenate([0.5 * Whz, 0.5 * Whr, 0.5 * Whn], axis=1),
                dtype=jnp.bfloat16,
            )
        )
    )
    brow = np.ascontiguousarray(
        np.concatenate([0.5 * b_iz, 0.5 * b_ir, 0.5 * b_hn, b_in])[None, :].astype(
            np.float32
        )
    )
    bin_ = np.ascontiguousarray(np.asarray(b_in, dtype=np.float32)[:, None])
    return wi, wh, brow, bin_


def make_in_maps(x, Wir, Wiz, Win, Whr, Whz, Whn, b_ir, b_iz, b_in, b_hn):
    x = np.ascontiguousarray(np.asarray(x, dtype=np.float32))
    bl = x.shape[0] // NCORES
    wi, wh, brow, bin_ = pack_weights(
        Wir, Wiz, Win, Whr, Whz, Whn, b_ir, b_iz, b_in, b_hn
    )
    return [
        {
            "x": x[i * bl : (i + 1) * bl],
            "wi": wi,
            "wh": wh,
            "b_row": brow,
            "b_in": bin_,
        }
        for i in range(NCORES)
    ]


def run_gru(x, Wir, Wiz, Win, Whr, Whz, Whn, b_ir, b_iz, b_in, b_hn, C=64, trace=False):
    x = np.ascontiguousarray(np.asarray(x, dtype=np.float32))
    Bx, Tx, Dx = x.shape
    bl = Bx // NCORES
    nc = _get_nc(bl, Tx, C)
    in_maps = make_in_maps(x, Wir, Wiz, Win, Whr, Whz, Whn, b_ir, b_iz, b_in, b_hn)
    res = run_bass_kernel_spmd(nc, in_maps, list(range(NCORES)), trace=trace)
    y = np.concatenate([res.results[i]["y"] for i in range(NCORES)], axis=0)
    return y, res


def kernel(**inputs) -> np.ndarray:
    inputs = {k: np.asarray(v) for k, v in inputs.items()}
    y, _ = run_gru(**inputs)
    return y.astype(np.float32)


if __name__ == "__main__":
    # smoke test with tiny T against a local numpy GRU reference
    rng = np.random.default_rng(0)
    Ts = 128
    s_i, s_h = 1.0 / np.sqrt(D), 1.0 / np.sqrt(H)
    inp = {
        "x": rng.standard_normal((B, Ts, D), dtype=np.float32),
        "Wir": rng.uniform(-s_i, s_i, (D, H)).astype(np.float32),
        "Wiz": rng.uniform(-s_i, s_i, (D, H)).astype(np.float32),
        "Win": rng.uniform(-s_i, s_i, (D, H)).astype(np.float32),
        "Whr": rng.uniform(-s_h, s_h, (H, H)).astype(np.float32),
        "Whz": rng.uniform(-s_h, s_h, (H, H)).astype(np.float32),
        "Whn": rng.uniform(-s_h, s_h, (H, H)).astype(np.float32),
        "b_ir": rng.uniform(-s_i, s_i, (H,)).astype(np.float32),
        "b_iz": rng.uniform(-s_i, s_i, (H,)).astype(np.float32),
        "b_in": rng.uniform(-s_i, s_i, (H,)).astype(np.float32),
        "b_hn": rng.uniform(-s_h, s_h, (H,)).astype(np.float32),
    }

    def np_gru(x, Wir, Wiz, Win, Whr, Whz, Whn, b_ir, b_iz, b_in, b_hn):
        Bx, Tx, _ = x.shape
        h = np.zeros((Bx, H), np.float32)
        gi_r = x @ Wir + b_ir
        gi_z = x @ Wiz + b_iz
        gi_n = x @ Win + b_in
        out = np.zeros((Bx, Tx, H), np.float32)
        for t in range(Tx):
            r = 1 / (1 + np.exp(-(gi_r[:, t] + h @ Whr)))
            z = 1 / (1 + np.exp(-(gi_z[:, t] + h @ Whz)))
            n = np.tanh(gi_n[:, t] + r * (h @ Whn + b_hn))
            h = (1 - z) * n + z * h
            out[:, t] = h
        return out

    expected = np_gru(**inp)
    y, _ = run_gru(**inp, C=64)
    err = np.abs(y - expected).max() / (np.abs(expected).max() + 1e-30)
    print("max abs err (rel to absmax):", err)
    assert err < 2e-3, err
    print("SMOKE TEST PASSED")
